# revision 1
# baseline (speedup 1.0000x reference)
"""Bass/Trainium2 kernel for nn_AttentionHead (B=4, C=D=64, H=W=64).

Sharding: 8 cores = 4 batches x 2 query-halves. Each core holds the full
x for its batch (keys/values need all 4096 positions) and computes
attention for 2048 query positions. Per-core inputs are column-rotated so
every core's query block is columns [0, 2048) of its own x — the program
is identical across cores (SPMD), only data differs. Softmax over keys is
permutation-invariant, so rotating the key order is exact.

On-device math (per core, N=4096 keys, NH=2048 queries):
  GroupNorm(num_groups=C) is affine per (batch, channel): xn = s*x + t with
  s = gn_w*rsqrt(var+eps), t = gn_b - mu*s. Folded into the projections:
  Q = (wq*s) @ x + (wq@t + bq), etc. An all-ones row appended to x makes
  every projection bias a plain matmul contraction (x_aug is [65, 4096]).
  Scores are computed transposed, S^T[m,n] = K[:,m]^T Q[:,n], so softmax'
  numerator exp(S^T/8) lands with keys on partitions — ready to be the
  moving operand of the AV matmul. No max-subtraction (|s/8| < ~2.5 here,
  exp is safe in fp32). The softmax denominator comes from an extra
  all-ones column in V' (so O'[64,n] = sum_m exp), and the final division
  is folded past the output projection: y = (wo@O + bo*denom) * (1/denom),
  with bo riding as row 64 of the augmented wo^T.
"""

import sys

sys.path.insert(0, "/opt/trn_rl_repo")

from contextlib import ExitStack

import numpy as np

import concourse.bass as bass
import concourse.tile as tile
from concourse import mybir
from concourse.bass import ts
from concourse.bass_utils import run_bass_kernel_spmd

# ---------------------------------------------------------------------------
# Workaround: this toolchain's walrus accepts at most ONE semaphore wait per
# instruction, but Tile's scheduler (and its tail drain) can attach several.
# Post-process every block, hoisting excess waits onto InstEventSemaphore
# carriers inserted immediately before the offending instruction on the same
# engine (engines execute their streams in order => semantically identical).
from concourse.vector_clock import ScopedClock as _ScopedClock
from concourse.bass import _bass_rust as _br


def _split_multiwait_instructions(nc, h0):
    cur_bb = nc.cur_bb.bb
    for f in nc.m.functions:
        for bb in f.blocks:
            insts = list(bb.instructions)
            out = []
            changed = False
            for ins in insts:
                si = ins.sync_info
                if si is not None:
                    waits = list(si.on_wait)
                    if len(waits) > 1:
                        for w in waits[:-1]:
                            carrier = nc.engines[ins.engine].wait_ge(h0, 0).ins
                            lst = list(cur_bb.instructions)
                            assert lst and lst[-1].name == carrier.name
                            lst.pop()
                            cur_bb.instructions = lst
                            carrier.sync_info.on_wait = [w]
                            out.append(carrier)
                        si.on_wait = [waits[-1]]
                        changed = True
                out.append(ins)
            if changed:
                bb.instructions = out


def _patched_drain_and_barrier(self, tick_clock, wait_clock):
    nc = self.nc
    assert self.sems is not None
    h0 = next(iter(self.sems.allocated().values()), None)
    if h0 is not None:
        _split_multiwait_instructions(nc, h0)

    drain_inst = nc.sync.drain()
    wait_clock.add_sem_waits(
        drain_inst.ins, _ScopedClock({None: tick_clock.global_clock})
    )
    si = drain_inst.ins.sync_info
    if si is not None:
        waits = list(si.on_wait)
        if len(waits) > 1:
            si.on_wait = [waits[0]]
            for w in waits[1:]:
                d2 = nc.sync.drain()
                _br.wait_op(d2.ins, h0, 0, "sem-ge", False)
                d2.ins.sync_info.on_wait = [w]

    nc.all_engine_barrier()
    popped = nc._tile_sem_poison_stack.pop()
    assert popped is self._sem_poison
    nc.clear_and_free_semaphores(list(self.sems.allocated().values()))
    nc.all_engine_barrier()


tile.TileContext._drain_and_barrier = _patched_drain_and_barrier
# ---------------------------------------------------------------------------

B, C, D, H, W = 4, 64, 64, 64, 64
N = H * W  # 4096 spatial positions (keys)
NCORES = 8
NH = N // 2  # 2048 queries per core
NT = 512  # query-tile width (one PSUM bank)
MT = 128  # key-tile height (matmul partition dim)
NJ = NH // NT  # 4 query tiles
NM = N // MT  # 32 key tiles
GRP = 3  # scores tiles per exp group (3 PSUM banks)
EPS = 1e-5
SCALE = 1.0 / np.sqrt(np.float32(D))  # folded into exp's free affine
F32 = mybir.dt.float32
F32R = mybir.dt.float32r  # TF32-like fast-fp32 matmul: 1 cyc/col (vs 4 for fp32)
                          # for moving dims >= 256; bitcast is zero-copy

_cache = {}


def _build_nc(reps=1):
    nc = bass.Bass()
    x_d = nc.declare_dram_parameter("x", [C, N], F32, isOutput=False)
    # all weights/biases packed into one tensor => one DMA:
    # cols [0:64] wqT + bq_row@64, [64:128] wkT + bk_row@64,
    # [128:192] wvT + bv_row@64, [192:256] woT + bo@64,
    # col 256 gn_weight (rows 0:64), col 257 gn_bias
    wp_d = nc.declare_dram_parameter("wpack", [C + 1, 258], F32, isOutput=False)
    out_d = nc.declare_dram_parameter("out", [D, NH], F32, isOutput=True)

    with tile.TileContext(nc) as tc, ExitStack() as ctx:
        consts = ctx.enter_context(tc.tile_pool(name="consts", bufs=1))
        big = ctx.enter_context(tc.tile_pool(name="big", bufs=1))
        exps = ctx.enter_context(tc.tile_pool(name="exps", bufs=3))
        outp = ctx.enter_context(tc.tile_pool(name="outp", bufs=3))
        ps_s = ctx.enter_context(tc.tile_pool(name="ps_s", bufs=2, space="PSUM"))
        ps_o = ctx.enter_context(tc.tile_pool(name="ps_o", bufs=2, space="PSUM"))

        with nc.allow_low_precision(reason="f32r (tf32) matmul operands"):
            for _rep in range(reps):
                _emit_body(nc, tc, consts, big, exps, outp, ps_s, ps_o,
                           x_d, wp_d, out_d)

    return nc


def _emit_body(nc, tc, consts, big, exps, outp, ps_s, ps_o, x_d, wp_d, out_d):
    if True:
        # warm the ACT table set (natural_log_exp) under the input DMAs so
        # the ~2.7us load is off the critical chain
        warm = consts.tile([1, 1], F32)
        nc.gpsimd.memset(warm, 0.0)
        nc.scalar.activation(
            out=warm, in_=warm, func=mybir.ActivationFunctionType.Exp
        )

        # ---- load weights / params: one packed DMA ---------------------
        dma = nc.default_dma_engine
        wp = consts.tile([C + 1, 258], F32)
        nc.gpsimd.dma_start(out=wp, in_=wp_d[:])
        wqT = wp[0:C, 0:D]
        wkT = wp[0:C, D : 2 * D]
        wvT = wp[0:C, 2 * D : 3 * D]
        woTa_f32 = wp[:, 3 * D : 4 * D]
        woTa = consts.tile([D + 1, D], F32R)
        nc.vector.tensor_copy(out=woTa, in_=woTa_f32)
        bq_row = wp[C : C + 1, 0:D]
        bk_row = wp[C : C + 1, D : 2 * D]
        bv_row = wp[C : C + 1, 2 * D : 3 * D]
        gnw = wp[0:C, 256:257]
        gnb = wp[0:C, 257:258]



        # ---- x with an all-ones row 64 ---------------------------------
        # chunked DMA so bn_stats can start on chunk 0 while later chunks
        # are still in flight
        x_aug = big.tile([C + 1, N], F32)
        stats = consts.tile([C, N // 512, 6], F32)
        dma_engines = [nc.sync, nc.scalar, nc.gpsimd]
        for q in range(8):
            dma_engines[q % 3].dma_start(
                out=x_aug[0:C, ts(q, 512)], in_=x_d[:, ts(q, 512)]
            )
            nc.vector.bn_stats(out=stats[:, q, :], in_=x_aug[0:C, ts(q, 512)])
        nc.gpsimd.memset(x_aug[C : C + 1, :], 1.0)
        # f32r-rounded copy for the fast-fp32 matmuls (ACT is idle here;
        # GroupNorm stats still read the exact fp32 x)
        x_r = big.tile([C + 1, N], F32R)
        for hh in range(2):
            nc.vector.tensor_copy(
                out=x_r[:, ts(hh, N // 2)], in_=x_aug[:, ts(hh, N // 2)]
            )
        ones_f32 = consts.tile([1, D], F32)
        nc.gpsimd.memset(ones_f32, 1.0)
        ones_col = consts.tile([1, D], F32R)
        nc.vector.tensor_copy(out=ones_col, in_=ones_f32)
        mv = consts.tile([C, 2], F32)
        nc.vector.bn_aggr(out=mv, in_=stats)

        # rs = (var+eps)^-0.5 via Ln/Exp (keeps everything in one ACT
        # table set, natural_log_exp_and_others, shared with the main exp)
        epst = consts.tile([C, 1], F32)
        nc.gpsimd.memset(epst, EPS)
        lnv = consts.tile([C, 1], F32)
        nc.scalar.activation(
            out=lnv, in_=mv[:, 1:2], func=mybir.ActivationFunctionType.Ln, bias=epst
        )
        rs = consts.tile([C, 1], F32)
        nc.scalar.activation(
            out=rs, in_=lnv, func=mybir.ActivationFunctionType.Exp, scale=-0.5
        )
        s_vec = consts.tile([C, 1], F32)
        nc.vector.tensor_mul(out=s_vec, in0=rs, in1=gnw)
        mus = consts.tile([C, 1], F32)
        nc.vector.tensor_mul(out=mus, in0=mv[:, 0:1], in1=s_vec)
        t_vec = consts.tile([C, 1], F32)
        nc.vector.tensor_sub(out=t_vec, in0=gnb, in1=mus)

        # ---- augmented projection weights ------------------------------
        # what_* rows 0..63 = w^T * s (per-channel), row 64 = (w@t + b)^T.
        # what_q2 is [wq_hat | wq_hat] so one matmul yields Q stacked twice
        # (rows 0:64 and 64:128) — the bottom copy feeds the row-packed
        # scores matmuls whose contraction lives on partitions 64..127.
        what_q2 = consts.tile([C + 1, 2 * D], F32R)
        what_k = consts.tile([C + 1, D], F32)
        what_k_r = consts.tile([C + 1, D], F32R)
        what_v = consts.tile([C + 1, D + 1], F32)
        nc.gpsimd.memset(what_v[:, D : D + 1], 0.0)
        nc.vector.tensor_scalar_mul(out=what_q2[0:C, 0:D], in0=wqT, scalar1=s_vec)
        nc.vector.tensor_scalar_mul(
            out=what_q2[0:C, D : 2 * D], in0=wqT, scalar1=s_vec
        )
        nc.vector.tensor_scalar_mul(out=what_k[0:C, :], in0=wkT, scalar1=s_vec)
        nc.vector.tensor_scalar_mul(out=what_k_r[0:C, :], in0=wkT, scalar1=s_vec)
        nc.vector.tensor_scalar_mul(out=what_v[0:C, 0:D], in0=wvT, scalar1=s_vec)
        for whT, wT, b_row, col0, col1 in (
            (what_q2, wqT, bq_row, 0, D),
            (what_q2, wqT, bq_row, D, 2 * D),
            (what_k, wkT, bk_row, 0, D),
            (what_k_r, wkT, bk_row, 0, D),
            (what_v, wvT, bv_row, 0, D),
        ):
            r_ps = ps_o.tile([MT, NT], F32, tag="o")
            nc.tensor.matmul(r_ps[0:1, 0:D], t_vec, wT, start=True, stop=True)
            nc.vector.tensor_add(
                out=whT[C : C + 1, col0:col1], in0=r_ps[0:1, 0:D], in1=b_row
            )
        nc.gpsimd.memset(what_v[C : C + 1, D : D + 1], 1.0)

        # ---- projections ----------------------------------------------
        # q2 [128, 2048]: Q duplicated on both partition halves.
        # k2 [128, 2048]: 512-col block t holds K of m-tiles {8t..8t+3} on
        # partitions 0:64 and {8t+4..8t+7} on 64:128 — each 128-col block
        # cb=4t+b pairs m-tiles (8t+b, 8t+4+b) for row-packed scores.
        # V'^T: 32 tiles of [128 keys, 65] (64 channels + ones column).
        k2_sb = big.tile([MT, N // 2], F32R)
        q2_sb = big.tile([MT, NH], F32R)
        vt_sb = big.tile([MT, NM * (D + 1)], F32R)

        def emit_q(j):
            p = ps_o.tile([MT, NT], F32, tag="o")
            nc.tensor.matmul(
                p, what_q2, x_r[:, ts(j, NT)], start=True, stop=True
            )
            nc.vector.tensor_copy(out=q2_sb[:, ts(j, NT)], in_=p)

        k_odd = big.tile([D, NH], F32R)

        def emit_k(t):
            # both chunks as fast f32r matmuls on partitions 0:64; the odd
            # chunk reaches k2's bottom partition half via a cross-partition
            # SBUF->SBUF DMA (f32r + col tile_position is rejected by the
            # compiler, and plain-fp32 matmuls are 4x slower)
            p = ps_o.tile([MT, NT], F32, tag="o")
            nc.tensor.matmul(
                p[0:D, 0:NT], what_k_r, x_r[:, ts(2 * t, NT)],
                start=True, stop=True,
            )
            nc.vector.tensor_copy(out=k2_sb[0:D, ts(t, NT)], in_=p[0:D, :])
            p2 = ps_o.tile([MT, NT], F32, tag="o")
            nc.tensor.matmul(
                p2[0:D, 0:NT], what_k_r, x_r[:, ts(2 * t + 1, NT)],
                start=True, stop=True,
            )
            nc.vector.tensor_copy(out=k_odd[:, ts(t, NT)], in_=p2[0:D, :])
            nc.gpsimd.dma_start(
                out=k2_sb[D:MT, ts(t, NT)], in_=k_odd[:, ts(t, NT)]
            )

        def emit_vt(m0, cnt):
            # 7 tiles of 65 columns per PSUM bank (455 <= 512); cnt <= 14
            p = ps_s.tile([MT, GRP * NT], F32, tag="sps")
            for k in range(cnt):
                off = (k // 7) * NT + (k % 7) * (D + 1)
                nc.tensor.matmul(
                    p[:, off : off + D + 1],
                    x_aug[:, ts(m0 + k, MT)],
                    what_v,
                    start=True, stop=True,
                )
            for b in range((cnt + 6) // 7):
                bc = min(7, cnt - 7 * b)
                nc.vector.tensor_copy(
                    out=vt_sb[
                        :, (m0 + 7 * b) * (D + 1) : (m0 + 7 * b + bc) * (D + 1)
                    ],
                    in_=p[:, b * NT : b * NT + bc * (D + 1)],
                )

        # pairs: K2 128-col block cb holds (m_top, m_bot); both scores
        # matmuls run concurrently in the PE array (row groups 0-1 / 2-3)
        pairs = [(8 * t + b, 8 * t + 4 + b, 4 * t + b) for t in range(4)
                 for b in range(4)]

        # global scores-tile q = 2*pair + half; exp groups of GRP tiles
        def tile_info(q):
            m_top, m_bot, cb = pairs[q // 2]
            if q % 2 == 0:
                return m_top, k2_sb[0:D, ts(cb, MT)], slice(0, D)
            return m_bot, k2_sb[D:MT, ts(cb, MT)], slice(D, MT)

        def emit_group(j, g, gsz, o_ps):
            s_ps = ps_s.tile([MT, GRP * NT], F32, tag="sps")
            infos = [tile_info(GRP * g + sl) for sl in range(gsz)]
            for sl, (mm, kq, qsl) in enumerate(infos):
                nc.tensor.matmul(
                    s_ps[:, ts(sl, NT)],
                    kq,
                    q2_sb[qsl, ts(j, NT)],
                    start=True, stop=True,
                )
            e_sb = exps.tile([MT, GRP * NT], F32R, tag="e")
            nc.scalar.activation(
                out=e_sb[:, 0 : gsz * NT],
                in_=s_ps[:, 0 : gsz * NT],
                func=mybir.ActivationFunctionType.Exp,
                scale=float(SCALE),
            )
            for sl, (mm, kq, qsl) in enumerate(infos):
                q = GRP * g + sl
                nc.tensor.matmul(
                    o_ps,
                    vt_sb[:, mm * (D + 1) : (mm + 1) * (D + 1)],
                    e_sb[:, ts(sl, NT)],
                    start=(q == 0),
                    stop=(q == 2 * len(pairs) - 1),
                )

        def emit_epilogue(j, o_ps):
            rec = outp.tile([1, NT], F32R, tag="rec")
            nc.vector.reciprocal(out=rec, in_=o_ps[D : D + 1, :])
            o_sb = outp.tile([D + 1, NT], F32R, tag="osb")
            nc.vector.tensor_copy(out=o_sb, in_=o_ps)
            rb_ps = ps_o.tile([D + 1, NT], F32, tag="o")
            nc.tensor.matmul(rb_ps[0:D, :], ones_col, rec, start=True, stop=True)
            rb_sb = outp.tile([D, NT], F32, tag="rb")
            nc.vector.tensor_copy(out=rb_sb, in_=rb_ps[0:D, :])
            z_ps = ps_o.tile([D + 1, NT], F32, tag="o")
            nc.tensor.matmul(z_ps[0:D, :], woTa, o_sb, start=True, stop=True)
            y_sb = outp.tile([D, NT], F32, tag="y")
            nc.vector.tensor_mul(out=y_sb, in0=z_ps[0:D, :], in1=rb_sb)
            dma.dma_start(out=out_d[:, ts(j, NT)], in_=y_sb)

        # ---- schedule --------------------------------------------------
        SCHED = globals().get("_SCHED", "interleave")
        GSIZES = [GRP] * (NM // GRP) + ([NM % GRP] if NM % GRP else [])
        if SCHED == "interleave":
            emit_q(0)
            emit_k(0)
            emit_k(1)
            emit_vt(0, 14)
            o_ps0 = ps_o.tile([D + 1, NT], F32, tag="o")
            for g in range(4):  # tiles 0..11: pairs 0-5 (k t0/t1, vt m<=13)
                emit_group(0, g, GSIZES[g], o_ps0)
            emit_q(1)
            emit_k(2)
            emit_vt(14, 14)
            for g in range(4, 8):  # tiles 12..23: pairs 6-11 (k t2, vt m<=27)
                emit_group(0, g, GSIZES[g], o_ps0)
            emit_q(2)
            emit_q(3)
            emit_k(3)
            emit_vt(28, 4)
            for g in range(8, len(GSIZES)):
                emit_group(0, g, GSIZES[g], o_ps0)
            emit_epilogue(0, o_ps0)
        else:
            for j in range(NJ):
                emit_q(j)
            for t in range(4):
                emit_k(t)
            for m0, cnt in ((0, 14), (14, 14), (28, 4)):
                emit_vt(m0, cnt)
            o_ps0 = ps_o.tile([D + 1, NT], F32, tag="o")
            for g in range(len(GSIZES)):
                emit_group(0, g, GSIZES[g], o_ps0)
            emit_epilogue(0, o_ps0)
        for j in range(1, NJ):
            o_ps = ps_o.tile([D + 1, NT], F32, tag="o")
            for g in range(len(GSIZES)):
                emit_group(j, g, GSIZES[g], o_ps)
            emit_epilogue(j, o_ps)


def _get_nc():
    if "nc" not in _cache:
        _cache["nc"] = _build_nc()
    return _cache["nc"]


class _Runner:
    """Cached SPMD executor: builds the shard_map'd jit once so repeat calls
    skip retracing (run_bass_via_pjrt rebuilds its jit on every call)."""

    def __init__(self, nc, n_cores):
        import jax
        from jax.sharding import Mesh, PartitionSpec
        from jax.experimental.shard_map import shard_map
        from concourse import bass2jax
        from concourse import mybir as _mb

        bass2jax.install_neuronx_cc_hook()
        partition_name = (
            nc.partition_id_tensor.name if nc.partition_id_tensor else None
        )
        in_names, out_names, out_avals, zero_outs = [], [], [], []
        for alloc in nc.m.functions[0].allocations:
            if not isinstance(alloc, _mb.MemoryLocationSet):
                continue
            name = alloc.memorylocations[0].name
            if alloc.kind == "ExternalInput":
                if name != partition_name:
                    in_names.append(name)
            elif alloc.kind == "ExternalOutput":
                out_names.append(name)
                shape = tuple(alloc.tensor_shape)
                dtype = _mb.dt.np(alloc.dtype)
                out_avals.append(jax.core.ShapedArray(shape, dtype))
                zero_outs.append(np.zeros(shape, dtype))
        self.in_names = list(in_names)
        self.out_names = list(out_names)
        self.out_avals = out_avals
        self.zero_outs = zero_outs
        n_params = len(in_names)
        all_in_names = in_names + out_names
        if partition_name is not None:
            all_in_names = all_in_names + [partition_name]

        def _body(*args):
            operands = list(args)
            if partition_name is not None:
                operands.append(bass2jax.partition_id_tensor())
            outs = bass2jax._bass_exec_p.bind(
                *operands,
                out_avals=tuple(out_avals),
                in_names=tuple(all_in_names),
                out_names=tuple(out_names),
                lowering_input_output_aliases=(),
                sim_require_finite=True,
                sim_require_nnan=True,
                nc=nc,
            )
            return tuple(outs)

        devices = jax.devices()[:n_cores]
        mesh = Mesh(np.asarray(devices), ("core",))
        n_outs = len(out_names)
        self.n_cores = n_cores
        self.fn = jax.jit(
            shard_map(
                _body,
                mesh=mesh,
                in_specs=(PartitionSpec("core"),) * (n_params + n_outs),
                out_specs=(PartitionSpec("core"),) * n_outs,
                check_rep=False,
            ),
            keep_unused=True,
        )

    def concat_inputs(self, in_maps):
        cat = [
            np.concatenate([m[name] for m in in_maps], axis=0)
            for name in self.in_names
        ]
        cat += [
            np.zeros((self.n_cores * z.shape[0], *z.shape[1:]), z.dtype)
            for z in self.zero_outs
        ]
        return cat

    def __call__(self, concat_in):
        return self.fn(*concat_in)

    def run(self, in_maps):
        import jax

        out_arrs = jax.block_until_ready(self(self.concat_inputs(in_maps)))
        return [
            {
                name: np.asarray(out_arrs[i]).reshape(
                    self.n_cores, *self.out_avals[i].shape
                )[c]
                for i, name in enumerate(self.out_names)
            }
            for c in range(self.n_cores)
        ]


def _get_runner():
    if "runner" not in _cache:
        _cache["runner"] = _Runner(_get_nc(), NCORES)
    return _cache["runner"]


def _make_in_maps(x, gn_weight, gn_bias, wq, bq, wk, bk, wv, bv, wo, bo):
    f = lambda a: np.ascontiguousarray(np.asarray(a, dtype=np.float32))
    x = f(x)
    wpack = np.zeros((C + 1, 258), dtype=np.float32)
    for i, (w, b) in enumerate(((wq, bq), (wk, bk), (wv, bv), (wo, bo))):
        wpack[0:C, i * D : (i + 1) * D] = f(w).T
        wpack[C, i * D : (i + 1) * D] = f(b)
    wpack[0:C, 256] = f(gn_weight)
    wpack[0:C, 257] = f(gn_bias)
    shared = {"wpack": wpack}
    in_maps = []
    for i in range(NCORES):
        b, h = divmod(i, 2)
        xb = x[b].reshape(C, N)
        if h:
            xb = np.concatenate([xb[:, NH:], xb[:, :NH]], axis=1)
        in_maps.append({"x": np.ascontiguousarray(xb), **shared})
    return in_maps


def kernel(x, gn_weight, gn_bias, wq, bq, wk, bk, wv, bv, wo, bo):
    in_maps = _make_in_maps(x, gn_weight, gn_bias, wq, bq, wk, bk, wv, bv, wo, bo)
    results = _get_runner().run(in_maps)
    out = np.empty((B, D, N), dtype=np.float32)
    for i in range(NCORES):
        b, h = divmod(i, 2)
        out[b, :, h * NH : (h + 1) * NH] = results[i]["out"]
    return out.reshape(B, D, H, W)

